# revision 39
# baseline (speedup 1.0000x reference)
"""Trainium2 Bass kernel for nn_AttentionSHA (dense transformer attention block).

Full inputs -> full output. Internally: tensor-parallel over heads across 8
NeuronCores (core g owns kv-head g and query heads 4g..4g+3; wo row-sharded),
host-side reduce of the 8 partial output projections (bf16 partials).

Performance: the QKV and WO projections run as fp8(e4m3) DoubleRow matmuls
(0.5 PE cycles per output row, 256-deep contraction per instruction) using a
hi+lo residual split of both operands for accuracy:
    a*b ~= a_hi*b_hi + (a_hi*b_lo + a_lo*b_hi)      [lo*lo dropped]
where v_hi = fp8(v), v_lo = fp8(v - v_hi) share one scale so all three
products accumulate in a single PSUM group: per pair of 128-deep contraction
tiles that is 3 DoubleRow instructions (vs 4 fp32r ones), i.e. 0.75x fp32r
cycles at ~10-bit effective mantissa (verified bit-exact vs ml_dtypes
emulation on device). The attention core (scores / exp / z / PV) stays fp32r.

DMA discipline: every DMACopy costs ~625ns on the shared HWDGE descriptor
generator in issue order, so transfers are batched into few chunk-contiguous
DMAs (x is laid out on host so each (s-half, m-group) chunk is one
contiguous 4KB/partition run).

Math notes (validated against the reference in fp64/fp32 numpy):
  - The reference adds a 0/1 causal mask *before* softmax (no -inf masking) and
    runs softmax over the full MAXSEQ=2048 cache axis where positions >= S hold
    zero k/v. Softmax without max-subtraction is exact here (scores are in
    [-17, 18]), so:  out = sum_t exp(sc_t)*m_t*v_t / (sum_t exp(sc_t)*m_t + 1024)
    with m_t = e if visible else 1, and +1024 = (MAXSEQ - S) zero-score tail.
    The e-factor for fully-visible regions folds into the Exp bias
    (exp(x + 1) = e*exp(x)); only the 128x128 diagonal blocks need a mask mult.
  - RoPE is applied via host-permuted weight rows (even channels then odd), a
    partition-half swap, and two multiply-adds against [cos;cos] / [-sin;sin].
  - Scales: wq/wk/wv stored *64 (cos/sin tables pre-divided by 64 unscale q,k
    in the rope); v flows *64 into PV, fixed by z' = (z+1024)*4 so the
    normalized att comes out *16 = the fp8 storage scale for the WO moving
    operand; wo stored *64; final psum *1024 -> output copy scales by 2^-10.
"""
import numpy as np
from contextlib import ExitStack

S = 1024
D = 4096
NH = 32
NKV = 8
HD = 128
NREP = NH // NKV          # 4
MAXSEQ = 2048
NCORES = 8
DT = D // 128             # 32 d-tiles
TT = S // 128             # 8 t-tiles
NP = DT // 2              # 16 d-tile pairs (DoubleRow steps)
NG = NP // 2              # 8 x-chunk groups per s-half (2 pairs each)

SW = 64.0                 # weight fp8 scale (wq/wk/wv/wo)
S8 = 16.0                 # att fp8 scale

_CACHE = {}


def _build_nc(dbg=False):
    import concourse.bacc as bacc
    import concourse.mybir as mybir
    import concourse.tile as tile

    f32 = mybir.dt.float32
    f32r = mybir.dt.float32r
    bf16 = mybir.dt.bfloat16
    f8 = mybir.dt.float8e4
    Exp = mybir.ActivationFunctionType.Exp
    Copy = mybir.ActivationFunctionType.Copy
    mult = mybir.AluOpType.mult
    add = mybir.AluOpType.add
    sub = mybir.AluOpType.subtract
    DR = mybir.MatmulPerfMode.DoubleRow

    nc = bacc.Bacc("TRN2", target_bir_lowering=False, debug=False,
                   num_devices=NCORES)

    # x hi/lo interleaved, chunk-contiguous:
    # [128, sh(2), grp(8), m2(2), sub(4), 512]; sub = (xl_a, xh_a, xl_b, xh_b)
    xc_d = nc.dram_tensor("xc", [128, 2 * NG * 2 * 4 * 512], f8,
                          kind="ExternalInput")
    # qkv weights hi/lo interleaved, sub-major so any subtile span is one
    # contiguous DMA: [128, 64 sub, 6 tensors (q0..q3,k,v), 128]
    wqkv_d = nc.dram_tensor("wqkv_c", [128, 64 * 6 * HD], f8,
                            kind="ExternalInput")
    # wo hi/lo interleaved over the head axis: [128, 8 sub, D]
    wo_d = nc.dram_tensor("wo_c", [128, 8 * D], f8, kind="ExternalInput")
    # merged constants: cc | ns | ident (f32); emaskd (bf16)
    cst_d = nc.dram_tensor("cst", [128, 2 * S + 128], f32, kind="ExternalInput")
    cst16_d = nc.dram_tensor("cst16", [128, TT * 128], bf16,
                             kind="ExternalInput")
    ones_d = nc.dram_tensor("ones", [128, 128], f32r, kind="ExternalInput")
    outT = nc.dram_tensor("outT", [D, S], bf16, kind="ExternalOutput")
    if dbg:
        dbg_d = {k: nc.dram_tensor(k, shp, dt, kind="ExternalOutput")
                 for k, shp, dt in [
                     ("qr0", [128, S], f32r), ("kr", [128, S], f32r),
                     ("vte", [128, S], f32r), ("ex00", [128, S], f32r),
                     ("ex05", [128, S], f32r), ("attc", [128, 8 * S], f8),
                     ("zz", [128, S], f32), ("rzz", [128, S], f32)]}

    with tile.TileContext(nc) as tc, ExitStack() as ctx:
        const = ctx.enter_context(tc.tile_pool(name="const", bufs=1))
        wts = ctx.enter_context(tc.tile_pool(name="wts", bufs=6))
        xpool = ctx.enter_context(tc.tile_pool(name="xpool", bufs=3))
        rpool = ctx.enter_context(tc.tile_pool(name="rpool", bufs=3))
        qkv = ctx.enter_context(tc.tile_pool(name="qkv", bufs=1))
        hs = ctx.enter_context(tc.tile_pool(name="hs", bufs=4))
        epool = ctx.enter_context(tc.tile_pool(name="epool", bufs=6))
        zpool = ctx.enter_context(tc.tile_pool(name="zpool", bufs=1))
        apool = ctx.enter_context(tc.tile_pool(name="apool", bufs=3))
        opool = ctx.enter_context(tc.tile_pool(name="opool", bufs=2))
        ps = ctx.enter_context(tc.tile_pool(name="ps", bufs=8, space="PSUM"))

        def _body():
            cst_sb = const.tile([128, 2 * S + 128], f32)
            cc_sb = cst_sb[:, 0:S]
            ns_sb = cst_sb[:, S:2 * S]
            ident_sb = cst_sb[:, 2 * S:]
            cst16_sb = const.tile([128, TT * 128], bf16)
            emaskd_sb = cst16_sb[:, 0:TT * 128]
            ones_sb = const.tile([128, 128], f32r)

            w_sb = wts.tile([128, 64, 6, HD], f8, name="w_sb", tag="w16",
                            bufs=1)

            def load_w_span(m0, mn):
                s0, s1 = 4 * m0, 4 * (m0 + mn)
                c0, c1 = s0 * 6 * HD, s1 * 6 * HD
                nc.sync.dma_start(
                    w_sb[:, s0:s1, :, :],
                    wqkv_d[:, c0:c1].rearrange("p (s t f) -> p s t f",
                                               t=6, f=HD))

            # just-in-time pair spans, one x-chunk group of lookahead; the
            # first span (pair 0) is hoisted before the loop for fast start
            _wb = {0: [(1, 1), (2, 2)]}
            _wb.update({g: [(2 * g + 2, 2)] for g in range(1, NG - 1)})

            # ---- phase 1: QKV projections (fp8 DoubleRow) + RoPE ----
            q_rot = [hs.tile([128, S], f32r, name=f"q_rot{h}", tag="hs")
                     for h in range(NREP)]                      # per head [e, s]
            k_rot = qkv.tile([128, S], f32r)                    # [e, t]
            v_et = qkv.tile([128, S], f32)                      # [e, t] pre-transpose
            v_te = qkv.tile([128, TT * 128], f32r)              # tile t: [t-part, e]
            inv_sqrt_hd = float(1.0 / np.sqrt(HD))

            # ---------- phase 3 pipeline helpers (used from sh=1 too) ----------
            expm_tiles = {}
            zo_ps = {}

            def emit_sc_exp(h, t, c):
                dlo, dhi = 128 * t, 128 * (t + 1)
                lo, hi = 512 * c, 512 * (c + 1)
                if (h, t) not in expm_tiles:
                    expm_tiles[(h, t)] = epool.tile([128, S], f32r, name="expm")
                expm = expm_tiles[(h, t)]
                sc = ps.tile([128, 512], f32, tag="ps", name="sc")
                nc.tensor.matmul(sc[:], k_rot[:, dlo:dhi],
                                 q_rot[h][:, lo:hi], start=True, stop=True)
                if dlo >= hi:
                    # fully invisible: plain exp
                    nc.scalar.activation(expm[:, lo:hi], sc[:], Exp,
                                         scale=inv_sqrt_hd)
                elif dhi <= lo:
                    # fully visible: exp(x + 1) = e * exp(x)
                    nc.scalar.activation(expm[:, lo:hi], sc[:], Exp,
                                         scale=inv_sqrt_hd, bias=1.0)
                else:
                    # diagonal block inside this chunk: one exp call, then the
                    # mask factors applied in-place (diag x emaskd on GpSimd;
                    # visible remainder x e on GpSimd)
                    nc.scalar.activation(expm[:, lo:hi], sc[:], Exp,
                                         scale=inv_sqrt_hd)
                    nc.gpsimd.tensor_tensor(
                        expm[:, dlo:dhi], expm[:, dlo:dhi],
                        emaskd_sb[:, 128 * t:128 * (t + 1)], op=mult)
                    if dhi < hi:
                        nc.gpsimd.tensor_scalar_mul(
                            expm[:, dhi:hi], expm[:, dhi:hi], float(np.e))

            att_c = apool.tile([128, 8, S], f8, name="att_c", tag="attc", bufs=1)
            z_sb = zpool.tile([128, S], f32, name="z_sb")
            rz = zpool.tile([128, S], f32, name="rz")

            def consume_zo(h, t, c):
                if (h, c) not in zo_ps:
                    zo_ps[(h, c)] = [ps.tile([128, 512], f32, tag="ps",
                                             name=f"zo{h}_{c}{i}")
                                     for i in range(2)]
                zp, op = zo_ps[(h, c)]
                cs = slice(512 * c, 512 * (c + 1))
                expm = expm_tiles[(h, t)]
                nc.tensor.matmul(zp[:], ones_sb[:], expm[:, cs],
                                 start=(t == 0), stop=(t == TT - 1))
                nc.tensor.matmul(op[:], v_te[:, 128 * t:128 * (t + 1)],
                                 expm[:, cs], start=(t == 0), stop=(t == TT - 1))
                if t == TT - 1:
                    # z' = (z + tail) * (SW/S8); att = o * (1/z') comes out *S8
                    nc.vector.tensor_scalar(z_sb[:, cs], zp[:],
                                            float(MAXSEQ - S), SW / S8,
                                            op0=add, op1=mult)
                    nc.vector.reciprocal(rz[:, cs], z_sb[:, cs])
                    at = apool.tile([128, 512], f32, name="at", tag="at")
                    nc.vector.tensor_tensor(at[:], op[:], rz[:, cs], op=mult)
                    # hi = fp8(at) on DVE; lo = at - hi on GpSimd
                    nc.vector.tensor_copy(att_c[:, 2 * h + 1, cs], at[:])
                    nc.gpsimd.tensor_tensor(att_c[:, 2 * h, cs], at[:],
                                            att_c[:, 2 * h + 1, cs], op=sub)
                    if dbg and h == 0 and c == 1:
                        nc.sync.dma_start(dbg_d["zz"][:], z_sb[:])
                        nc.sync.dma_start(dbg_d["rzz"][:], rz[:])
                if dbg and (h, t) == (0, 0) and c == 1:
                    nc.sync.dma_start(dbg_d["ex00"][:], expm[:])
                if dbg and (h, t) == (0, 5) and c == 1:
                    nc.sync.dma_start(dbg_d["ex05"][:], expm[:])

            def transpose_v(t):
                tr = ps.tile([128, 128], f32, tag="ps", name="tr")
                nc.tensor.transpose(tr[:], v_et[:, 128 * t:128 * (t + 1)],
                                    ident_sb[:])
                nc.vector.tensor_copy(v_te[:, 128 * t:128 * (t + 1)], tr[:])

            # jobs: head 0 c=0 for t<4 is feasible right after sh=0 rope --
            # those four (the seeds) are emitted inside the sh=1 DR stream.
            jobs = [(0, t, 0) for t in range(4)]
            for h in range(NREP):
                for t in range(TT):
                    for c in range(2):
                        if (h, t, c) not in jobs:
                            jobs.append((h, t, c))

            # interleaved into the sh=1 stream at m-group boundaries:
            # [transpose_v(0..3) at g=0,1 | seeds at g=2..5]
            def sh1_filler(g):
                if g < 2:
                    transpose_v(2 * g)
                    transpose_v(2 * g + 1)
                elif g < 6:
                    emit_sc_exp(*jobs[g - 2])

            # x chunks are DMA'd two groups ahead of consumption
            x_tiles = {}

            def prime_x(idx):
                if idx >= 2 * NG:
                    return
                x_r = x_tiles[idx] = xpool.tile([128, 2, 4, 512], f8,
                                                name="x_r")
                nc.sync.dma_start(
                    x_r[:],
                    xc_d[:, idx * 4096:(idx + 1) * 4096].rearrange(
                        "p (m s f) -> p m s f", m=2, f=512))

            prime_x(0)
            load_w_span(0, 1)
            prime_x(1)
            for sh in range(2):
                s0 = 512 * sh
                q_ps = [ps.tile([128, 512], f32, tag="ps", name=f"q_ps{sh}_{h}")
                        for h in range(NREP)]
                k_ps = ps.tile([128, 512], f32, tag="ps", name=f"k_ps{sh}")
                v_ps = ps.tile([128, 512], f32, tag="ps", name=f"v_ps{sh}")
                for g in range(NG):
                    x_r = x_tiles.pop(sh * NG + g)
                    prime_x(sh * NG + g + 2)
                    if sh == 0 and g in _wb:
                        for span in _wb[g]:
                            load_w_span(*span)
                    if sh == 0 and g == 6:
                        nc.sync.dma_start(cst_sb[:], cst_d[:])
                    if sh == 1 and g == 0:
                        nc.sync.dma_start(cst16_sb[:], cst16_d[:])
                        nc.sync.dma_start(ones_sb[:], ones_d[:])
                    for j in range(2):
                        m = 2 * g + j
                        st = (m == 0)
                        sp = (m == NP - 1)
                        for ti, dst in enumerate(q_ps + [k_ps, v_ps]):
                            # hi*hi for both d-tiles of the pair
                            nc.tensor.matmul(dst[:],
                                             w_sb[:, 4 * m:4 * m + 3:2, ti, :],
                                             x_r[:, j, 1:4:2, :],
                                             start=st, stop=False, perf_mode=DR)
                            # per d-tile: w_hi*x_lo + w_lo*x_hi
                            nc.tensor.matmul(dst[:],
                                             w_sb[:, 4 * m:4 * m + 2, ti, :],
                                             x_r[:, j, 0:2, :],
                                             start=False, stop=False,
                                             perf_mode=DR)
                            nc.tensor.matmul(dst[:],
                                             w_sb[:, 4 * m + 2:4 * m + 4, ti, :],
                                             x_r[:, j, 2:4, :],
                                             start=False, stop=sp, perf_mode=DR)
                    if sh == 1:
                        sh1_filler(g)

                # RoPE: dest = psum*[cos;cos] + swap(psum)*[-sin;sin].
                # fast=True splits the swap copies across ACT+DVE — used on
                # sh=1 where rope latency gates the phase-3 pipeline restart
                def rope(psum, dest, fast=False):
                    sw = rpool.tile([128, 512], f32, name="sw")
                    if fast:
                        nc.vector.tensor_copy(sw[0:64, :], psum[64:128, :])
                    else:
                        nc.scalar.copy(sw[0:64, :], psum[64:128, :])
                    nc.scalar.copy(sw[64:128, :], psum[0:64, :])
                    t1 = rpool.tile([128, 512], f32, name="t1")
                    nc.vector.tensor_tensor(t1[:], psum[:], cc_sb[:, s0:s0 + 512], op=mult)
                    t2 = rpool.tile([128, 512], f32, name="t2")
                    nc.gpsimd.tensor_tensor(t2[:], sw[:], ns_sb[:, s0:s0 + 512], op=mult)
                    nc.vector.tensor_tensor(dest, t1[:], t2[:], op=add)

                nc.vector.tensor_copy(v_et[:, s0:s0 + 512], v_ps[:])
                rope(q_ps[0], q_rot[0][:, s0:s0 + 512], fast=(sh == 1))
                rope(k_ps, k_rot[:, s0:s0 + 512], fast=(sh == 1))
                for h in range(1, NREP):
                    rope(q_ps[h], q_rot[h][:, s0:s0 + 512], fast=(sh == 1))

            # ---- phase 3: attention, flat (h, t, c) pipeline ----
            # wo for phase 4 loads during phase 3 (4MB, off the critical path)
            wo_sb = wts.tile([128, 8, D], f8, name="wo_sb", tag="wo", bufs=1)
            nc.sync.dma_start(wo_sb[:],
                              wo_d[:].rearrange("p (s f) -> p s f", f=D))

            # s-half-1 V transposes (feed o-matmuls for t>=4, reached later)
            for t in range(4, TT):
                transpose_v(t)

            PREFILL = 5
            emitted, consumed = 4, 0           # 4 seeds already emitted
            while consumed < len(jobs):
                if emitted < len(jobs) and emitted - consumed <= PREFILL:
                    emit_sc_exp(*jobs[emitted])
                    emitted += 1
                else:
                    consume_zo(*jobs[consumed])
                    consumed += 1

            if dbg:
                nc.sync.dma_start(dbg_d["qr0"][:], q_rot[0][:])
                nc.sync.dma_start(dbg_d["kr"][:], k_rot[:])
                nc.sync.dma_start(dbg_d["vte"][:], v_te[:])
                nc.sync.dma_start(
                    dbg_d["attc"][:],
                    att_c[:].rearrange("p a b -> p (a b)"))

            # ---- phase 4: output projection (fp8 DoubleRow over this core's
            # 512 att channels; psum is *S8*SW = 1024x, output copy scales back)
            for do in range(DT):
                dc = slice(128 * do, 128 * (do + 1))
                out_sb = opool.tile([128, S], bf16, name="out_sb")
                op_ps = [ps.tile([128, 512], f32, tag="ps", name=f"op{c}")
                         for c in range(2)]
                for c in range(2):
                    cs = slice(512 * c, 512 * (c + 1))
                    # per head: wo_hi*att_lo + wo_lo*att_hi
                    for hh in range(NREP):
                        nc.tensor.matmul(op_ps[c][:],
                                         wo_sb[:, 2 * hh:2 * hh + 2, dc],
                                         att_c[:, 2 * hh:2 * hh + 2, cs],
                                         start=(hh == 0), stop=False,
                                         perf_mode=DR)
                    # hi*hi for head pairs (0,1) and (2,3)
                    nc.tensor.matmul(op_ps[c][:], wo_sb[:, 0:3:2, dc],
                                     att_c[:, 1:4:2, cs],
                                     start=False, stop=False, perf_mode=DR)
                    nc.tensor.matmul(op_ps[c][:], wo_sb[:, 4:7:2, dc],
                                     att_c[:, 5:8:2, cs],
                                     start=False, stop=True, perf_mode=DR)
                    # copy each chunk as soon as its psum group closes
                    if c == 0:
                        nc.vector.tensor_scalar_mul(out_sb[:, 0:512],
                                                    op_ps[0][:],
                                                    1.0 / (S8 * SW))
                    else:
                        nc.scalar.activation(out_sb[:, 512:1024], op_ps[1][:],
                                             Copy, scale=1.0 / (S8 * SW))
                nc.sync.dma_start(outT[dc, :], out_sb[:])

        _body()

    nc.compile()
    return nc


def _to_f32r(x):
    """Host replica of the device fp32 -> fp32r conversion: round-to-nearest-
    even to an 11-bit mantissa (low 12 bits zeroed). Verified bit-exact against
    the DVE/DMA converters."""
    xi = np.ascontiguousarray(x, np.float32).view(np.uint32).astype(np.uint64)
    r = ((xi + 0x7FF + ((xi >> 12) & 1)) >> 12) << 12
    return (r & 0xFFFFFFFF).astype(np.uint32).view(np.float32)


def _split8(x):
    """fp8(e4m3) hi + lo residual split, shared scale. Returns (hi, lo)."""
    import ml_dtypes
    E4 = ml_dtypes.float8_e4m3
    hi = np.asarray(x, np.float32).astype(E4)
    lo = (np.asarray(x, np.float32) - hi.astype(np.float32)).astype(E4)
    return hi, lo


def ml_dtypes_bf16():
    import ml_dtypes
    return ml_dtypes.bfloat16


def kernel(**inputs):
    from concourse.bass_utils import run_bass_kernel_spmd

    x = np.asarray(inputs["x"], np.float32)                 # [1, S, D]
    cos = np.asarray(inputs["freqs_cos"], np.float32)       # [S, 64]
    sin = np.asarray(inputs["freqs_sin"], np.float32)       # [S, 64]
    wq = np.asarray(inputs["wq"], np.float32)               # [NH, HD, D]
    wk = np.asarray(inputs["wk"], np.float32)               # [NKV, HD, D]
    wv = np.asarray(inputs["wv"], np.float32)               # [NKV, HD, D]
    wo = np.asarray(inputs["wo"], np.float32)               # [D, D]
    input_pos = np.asarray(inputs["input_pos"]).astype(np.int64)  # [S]

    if "nc" not in _CACHE:
        _CACHE["nc"] = _build_nc()
    nc = _CACHE["nc"]

    perm = np.concatenate([np.arange(0, HD, 2), np.arange(1, HD, 2)])

    # x: [D, S] -> [p, sh, grp, m2, sub4, 512] with sub = (xl_a, xh_a, xl_b, xh_b)
    xT = x[0].T                                             # [D, S]
    xh, xl = _split8(xT)
    x4 = np.empty((DT, 2, 128, S), dtype=xh.dtype)          # [d-tile, lo/hi, p, s]
    x4[:, 1] = xh.reshape(DT, 128, S)
    x4[:, 0] = xl.reshape(DT, 128, S)
    # [d-tile, l/h, p, s] -> [m, m2, sub-lh, p, sh, 512] -> [p, sh, m, m2, sub, 512]
    xc = np.ascontiguousarray(
        x4.reshape(NP, 2, 2, 128, 2, 512)                   # [m, m2, lh, p, sh, 512]
        .transpose(3, 4, 0, 1, 2, 5)                        # [p, sh, m, m2, lh, 512]
        .reshape(128, 2 * NP * 2 * 2 * 512))
    # note: within a pair m the four subtiles must be (xl_a, xh_a, xl_b, xh_b)
    # -> order above is [m2(pair member), lh(lo,hi)] which flattens exactly so.

    def wsplit(wT):
        # [D, 128e] -> hi/lo interleaved [128p, 64 sub, 128e]
        h8, l8 = _split8(wT * SW)
        out = np.empty((128, 64, HD), dtype=h8.dtype)
        out[:, 0::2, :] = h8.reshape(DT, 128, HD).transpose(1, 0, 2)
        out[:, 1::2, :] = l8.reshape(DT, 128, HD).transpose(1, 0, 2)
        return out

    # cos/sin tables absorb the 1/SW unscale of q,k
    cc = np.ascontiguousarray(np.concatenate([cos.T, cos.T], 0)) / SW   # [128, S]
    ns = np.ascontiguousarray(np.concatenate([-sin.T, sin.T], 0)) / SW  # [128, S]
    # visibility adds +1 pre-exp where input_pos[t] <= input_pos[s]; for the
    # (spec-guaranteed) sorted arange fill only diagonal blocks are mixed.
    emaskd_t = np.empty((TT, 128, 128), np.float32)
    for t in range(TT):
        p = input_pos[128 * t:128 * (t + 1)]
        emaskd_t[t] = np.where(p[:, None] <= p[None, :], np.float32(np.e),
                               np.float32(1.0))
    emaskd = np.ascontiguousarray(
        emaskd_t.transpose(1, 0, 2).reshape(128, TT * 128))
    ident = np.eye(128, dtype=np.float32)
    cst = np.concatenate([cc, ns, ident], axis=1)
    cst16 = emaskd.astype(ml_dtypes_bf16())
    ones128 = np.ones((128, 128), np.float32)

    in_maps = []
    for g in range(NCORES):
        wq_g = wq[NREP * g:NREP * (g + 1)][:, perm, :]       # [4, 128, D]

        # wo rows for this core's 4 heads, hi/lo interleaved [128, 8 sub, D]
        wo_g = wo[:, NREP * HD * g:NREP * HD * (g + 1)].T    # [512, D]
        wh8, wl8 = _split8(wo_g * SW)
        wo_c = np.empty((128, 8, D), dtype=wh8.dtype)
        wo_c[:, 0::2, :] = wh8.reshape(NREP, 128, D).transpose(1, 0, 2)
        wo_c[:, 1::2, :] = wl8.reshape(NREP, 128, D).transpose(1, 0, 2)

        # [tensor(6), 128p, 64 sub, 128e] -> sub-major [128p, 64, 6, 128]
        wqkv = np.stack([wsplit(wq_g[j].T) for j in range(NREP)]
                        + [wsplit(wk[g][perm].T), wsplit(wv[g].T)])
        wqkv = np.ascontiguousarray(wqkv.transpose(1, 2, 0, 3))

        in_maps.append({
            "xc": xc,
            "wqkv_c": wqkv.reshape(128, 64 * 6 * HD),
            "wo_c": wo_c.reshape(128, 8 * D),
            "cst": cst,
            "cst16": cst16,
            "ones": _to_f32r(ones128),
        })

    res = run_bass_kernel_spmd(nc, in_maps, list(range(NCORES)))
    total = np.zeros((D, S), np.float32)
    for g in range(NCORES):
        total += res.results[g]["outT"].astype(np.float32)
    return np.ascontiguousarray(total.T[None])   # [1, S, D]


# revision 42
# speedup vs baseline: 1.1170x; 1.1170x over previous
"""Trainium2 Bass kernel for nn_AttentionSHA (dense transformer attention block).

Full inputs -> full output. Internally: tensor-parallel over heads across 8
NeuronCores (core g owns kv-head g and query heads 4g..4g+3; wo row-sharded),
host-side reduce of the 8 partial output projections (bf16 partials).

Performance: the QKV and WO projections run as fp8(e4m3) DoubleRow matmuls
(0.5 PE cycles per output row, 256-deep contraction per instruction) using a
hi+lo residual split of both operands for accuracy:
    a*b ~= a_hi*b_hi + (a_hi*b_lo + a_lo*b_hi)      [lo*lo dropped]
where v_hi = fp8(v), v_lo = fp8(v - v_hi) share one scale so all three
products accumulate in a single PSUM group: per pair of 128-deep contraction
tiles that is 3 DoubleRow instructions (vs 4 fp32r ones), i.e. 0.75x fp32r
cycles at ~10-bit effective mantissa (verified bit-exact vs ml_dtypes
emulation on device). The attention core (scores / exp / z / PV) stays fp32r.

DMA discipline: every DMACopy costs ~625ns on the shared HWDGE descriptor
generator in issue order, so transfers are batched into few chunk-contiguous
DMAs (x is laid out on host so each (s-half, m-group) chunk is one
contiguous 4KB/partition run).

Math notes (validated against the reference in fp64/fp32 numpy):
  - The reference adds a 0/1 causal mask *before* softmax (no -inf masking) and
    runs softmax over the full MAXSEQ=2048 cache axis where positions >= S hold
    zero k/v. Softmax without max-subtraction is exact here (scores are in
    [-17, 18]), so:  out = sum_t exp(sc_t)*m_t*v_t / (sum_t exp(sc_t)*m_t + 1024)
    with m_t = e if visible else 1, and +1024 = (MAXSEQ - S) zero-score tail.
    The e-factor for fully-visible regions folds into the Exp bias
    (exp(x + 1) = e*exp(x)); only the 128x128 diagonal blocks need a mask mult.
  - RoPE is applied via host-permuted weight rows (even channels then odd), a
    partition-half swap, and two multiply-adds against [cos;cos] / [-sin;sin].
  - Scales: wq/wk/wv stored *64 (cos/sin tables pre-divided by 64 unscale q,k
    in the rope); v flows *64 into PV, fixed by z' = (z+1024)*4 so the
    normalized att comes out *16 = the fp8 storage scale for the WO moving
    operand; wo stored *64; final psum *1024 -> output copy scales by 2^-10.
"""
import numpy as np
from contextlib import ExitStack

S = 1024
D = 4096
NH = 32
NKV = 8
HD = 128
NREP = NH // NKV          # 4
MAXSEQ = 2048
NCORES = 8
DT = D // 128             # 32 d-tiles
TT = S // 128             # 8 t-tiles
NP = DT // 2              # 16 d-tile pairs (DoubleRow steps)
NG = NP // 2              # 8 x-chunk groups per s-half (2 pairs each)

SW = 64.0                 # weight fp8 scale (wq/wk/wv/wo)
S8 = 16.0                 # att fp8 scale

_CACHE = {}


def _build_nc(dbg=False):
    import concourse.bacc as bacc
    import concourse.mybir as mybir
    import concourse.tile as tile

    f32 = mybir.dt.float32
    f32r = mybir.dt.float32r
    bf16 = mybir.dt.bfloat16
    f8 = mybir.dt.float8e4
    Exp = mybir.ActivationFunctionType.Exp
    Copy = mybir.ActivationFunctionType.Copy
    mult = mybir.AluOpType.mult
    add = mybir.AluOpType.add
    sub = mybir.AluOpType.subtract
    DR = mybir.MatmulPerfMode.DoubleRow

    nc = bacc.Bacc("TRN2", target_bir_lowering=False, debug=False,
                   num_devices=NCORES)

    # x hi/lo interleaved, chunk-contiguous:
    # [128, sh(2), grp(8), m2(2), sub(4), 512]; sub = (xl_a, xh_a, xl_b, xh_b)
    xc_d = nc.dram_tensor("xc", [128, 2 * NG * 2 * 4 * 512], f8,
                          kind="ExternalInput")
    # qkv weights hi/lo interleaved, sub-major so any subtile span is one
    # contiguous DMA: [128, 64 sub, 6 tensors (q0..q3,k,v), 128]
    wqkv_d = nc.dram_tensor("wqkv_c", [128, 64 * 6 * HD], f8,
                            kind="ExternalInput")
    # wo hi/lo interleaved over the head axis: [128, 8 sub, D]
    wo_d = nc.dram_tensor("wo_c", [128, 8 * D], f8, kind="ExternalInput")
    # merged constants: cc | ns | ident (f32); emaskd (bf16)
    cst_d = nc.dram_tensor("cst", [128, 2 * S + 128], f32, kind="ExternalInput")
    cst16_d = nc.dram_tensor("cst16", [128, TT * 128], bf16,
                             kind="ExternalInput")
    ones_d = nc.dram_tensor("ones", [128, 128], f32r, kind="ExternalInput")
    outT = nc.dram_tensor("outT", [D, S], bf16, kind="ExternalOutput")
    if dbg:
        dbg_d = {k: nc.dram_tensor(k, shp, dt, kind="ExternalOutput")
                 for k, shp, dt in [
                     ("qr0", [128, S], f32r), ("kr", [128, S], f32r),
                     ("vte", [128, S], f32r), ("ex00", [128, S], f32r),
                     ("ex05", [128, S], f32r), ("attc", [128, 8 * S], f8),
                     ("zz", [128, S], f32), ("rzz", [128, S], f32)]}

    with tile.TileContext(nc) as tc, ExitStack() as ctx:
        const = ctx.enter_context(tc.tile_pool(name="const", bufs=1))
        wts = ctx.enter_context(tc.tile_pool(name="wts", bufs=6))
        xpool = ctx.enter_context(tc.tile_pool(name="xpool", bufs=3))
        rpool = ctx.enter_context(tc.tile_pool(name="rpool", bufs=3))
        qkv = ctx.enter_context(tc.tile_pool(name="qkv", bufs=1))
        hs = ctx.enter_context(tc.tile_pool(name="hs", bufs=4))
        epool = ctx.enter_context(tc.tile_pool(name="epool", bufs=6))
        zpool = ctx.enter_context(tc.tile_pool(name="zpool", bufs=1))
        apool = ctx.enter_context(tc.tile_pool(name="apool", bufs=3))
        opool = ctx.enter_context(tc.tile_pool(name="opool", bufs=3))
        ps = ctx.enter_context(tc.tile_pool(name="ps", bufs=8, space="PSUM"))

        def _body():
            cst_sb = const.tile([128, 2 * S + 128], f32)
            cc_sb = cst_sb[:, 0:S]
            ns_sb = cst_sb[:, S:2 * S]
            ident_sb = cst_sb[:, 2 * S:]
            cst16_sb = const.tile([128, TT * 128], bf16)
            emaskd_sb = cst16_sb[:, 0:TT * 128]
            ones_sb = const.tile([128, 128], f32r)

            w_sb = wts.tile([128, 64, 6, HD], f8, name="w_sb", tag="w16",
                            bufs=1)

            def load_w_span(m0, mn):
                s0, s1 = 4 * m0, 4 * (m0 + mn)
                c0, c1 = s0 * 6 * HD, s1 * 6 * HD
                nc.sync.dma_start(
                    w_sb[:, s0:s1, :, :],
                    wqkv_d[:, c0:c1].rearrange("p (s t f) -> p s t f",
                                               t=6, f=HD))

            # just-in-time pair spans, one x-chunk group of lookahead; the
            # first span (pair 0) is hoisted before the loop for fast start
            _wb = {0: [(1, 1), (2, 2)]}
            _wb.update({g: [(2 * g + 2, 2)] for g in range(1, NG - 1)})

            # ---- phase 1: QKV projections (fp8 DoubleRow) + RoPE ----
            q_rot = [hs.tile([128, S], f32r, name=f"q_rot{h}", tag="hs")
                     for h in range(NREP)]                      # per head [e, s]
            k_rot = qkv.tile([128, S], f32r)                    # [e, t]
            v_et = qkv.tile([128, S], f32)                      # [e, t] pre-transpose
            v_te = qkv.tile([128, TT * 128], f32r)              # tile t: [t-part, e]
            inv_sqrt_hd = float(1.0 / np.sqrt(HD))

            # ---------- phase 3 pipeline helpers (used from sh=1 too) ----------
            expm_tiles = {}
            zo_ps = {}

            def emit_sc_exp(h, t, c):
                dlo, dhi = 128 * t, 128 * (t + 1)
                lo, hi = 512 * c, 512 * (c + 1)
                if (h, t) not in expm_tiles:
                    expm_tiles[(h, t)] = epool.tile([128, S], f32r, name="expm")
                expm = expm_tiles[(h, t)]
                sc = ps.tile([128, 512], f32, tag="ps", name="sc")
                nc.tensor.matmul(sc[:], k_rot[:, dlo:dhi],
                                 q_rot[h][:, lo:hi], start=True, stop=True)
                if dlo >= hi:
                    # fully invisible: plain exp
                    nc.scalar.activation(expm[:, lo:hi], sc[:], Exp,
                                         scale=inv_sqrt_hd)
                elif dhi <= lo:
                    # fully visible: exp(x + 1) = e * exp(x)
                    nc.scalar.activation(expm[:, lo:hi], sc[:], Exp,
                                         scale=inv_sqrt_hd, bias=1.0)
                else:
                    # diagonal block inside this chunk: one exp call, then the
                    # mask factors applied in-place (diag x emaskd on GpSimd;
                    # visible remainder x e on GpSimd)
                    nc.scalar.activation(expm[:, lo:hi], sc[:], Exp,
                                         scale=inv_sqrt_hd)
                    nc.gpsimd.tensor_tensor(
                        expm[:, dlo:dhi], expm[:, dlo:dhi],
                        emaskd_sb[:, 128 * t:128 * (t + 1)], op=mult)
                    if dhi < hi:
                        nc.gpsimd.tensor_scalar_mul(
                            expm[:, dhi:hi], expm[:, dhi:hi], float(np.e))

            att_c = apool.tile([128, 8, S], f8, name="att_c", tag="attc", bufs=1)
            z_sb = zpool.tile([128, S], f32, name="z_sb")
            rz = zpool.tile([128, S], f32, name="rz")

            def consume_zo(h, t, c):
                if (h, c) not in zo_ps:
                    zo_ps[(h, c)] = [ps.tile([128, 512], f32, tag="ps",
                                             name=f"zo{h}_{c}{i}")
                                     for i in range(2)]
                zp, op = zo_ps[(h, c)]
                cs = slice(512 * c, 512 * (c + 1))
                expm = expm_tiles[(h, t)]
                nc.tensor.matmul(zp[:], ones_sb[:], expm[:, cs],
                                 start=(t == 0), stop=(t == TT - 1))
                nc.tensor.matmul(op[:], v_te[:, 128 * t:128 * (t + 1)],
                                 expm[:, cs], start=(t == 0), stop=(t == TT - 1))
                if t == TT - 1:
                    # z' = (z + tail) * (SW/S8); att = o * (1/z') comes out *S8
                    nc.vector.tensor_scalar(z_sb[:, cs], zp[:],
                                            float(MAXSEQ - S), SW / S8,
                                            op0=add, op1=mult)
                    nc.vector.reciprocal(rz[:, cs], z_sb[:, cs])
                    at = apool.tile([128, 512], f32, name="at", tag="at")
                    nc.vector.tensor_tensor(at[:], op[:], rz[:, cs], op=mult)
                    # hi = fp8(at) on DVE; lo = at - hi on GpSimd
                    nc.vector.tensor_copy(att_c[:, 2 * h + 1, cs], at[:])
                    nc.gpsimd.tensor_tensor(att_c[:, 2 * h, cs], at[:],
                                            att_c[:, 2 * h + 1, cs], op=sub)
                    if dbg and h == 0 and c == 1:
                        nc.sync.dma_start(dbg_d["zz"][:], z_sb[:])
                        nc.sync.dma_start(dbg_d["rzz"][:], rz[:])
                if dbg and (h, t) == (0, 0) and c == 1:
                    nc.sync.dma_start(dbg_d["ex00"][:], expm[:])
                if dbg and (h, t) == (0, 5) and c == 1:
                    nc.sync.dma_start(dbg_d["ex05"][:], expm[:])

            def transpose_v(t):
                tr = ps.tile([128, 128], f32, tag="ps", name="tr")
                nc.tensor.transpose(tr[:], v_et[:, 128 * t:128 * (t + 1)],
                                    ident_sb[:])
                nc.vector.tensor_copy(v_te[:, 128 * t:128 * (t + 1)], tr[:])

            # jobs: head 0 c=0 for t<4 is feasible right after sh=0 rope --
            # those four (the seeds) are emitted inside the sh=1 DR stream.
            jobs = [(0, t, 0) for t in range(4)]
            for h in range(NREP):
                for t in range(TT):
                    for c in range(2):
                        if (h, t, c) not in jobs:
                            jobs.append((h, t, c))

            # interleaved into the sh=1 stream at m-group boundaries:
            # [transpose_v(0..3) at g=0,1 | seeds at g=2..5]
            def sh1_filler(g):
                if g < 2:
                    transpose_v(2 * g)
                    transpose_v(2 * g + 1)
                elif g < 6:
                    emit_sc_exp(*jobs[g - 2])

            # x chunks are DMA'd two groups ahead of consumption
            x_tiles = {}

            def prime_x(idx):
                if idx >= 2 * NG:
                    return
                x_r = x_tiles[idx] = xpool.tile([128, 2, 4, 512], f8,
                                                name="x_r")
                nc.sync.dma_start(
                    x_r[:],
                    xc_d[:, idx * 4096:(idx + 1) * 4096].rearrange(
                        "p (m s f) -> p m s f", m=2, f=512))

            prime_x(0)
            load_w_span(0, 1)
            prime_x(1)
            for sh in range(2):
                s0 = 512 * sh
                q_ps = [ps.tile([128, 512], f32, tag="ps", name=f"q_ps{sh}_{h}")
                        for h in range(NREP)]
                k_ps = ps.tile([128, 512], f32, tag="ps", name=f"k_ps{sh}")
                v_ps = ps.tile([128, 512], f32, tag="ps", name=f"v_ps{sh}")
                for g in range(NG):
                    x_r = x_tiles.pop(sh * NG + g)
                    prime_x(sh * NG + g + 2)
                    if sh == 0 and g in _wb:
                        for span in _wb[g]:
                            load_w_span(*span)
                    if sh == 0 and g == 6:
                        nc.sync.dma_start(cst_sb[:], cst_d[:])
                    if sh == 1 and g == 0:
                        nc.sync.dma_start(cst16_sb[:], cst16_d[:])
                        nc.sync.dma_start(ones_sb[:], ones_d[:])
                    for j in range(2):
                        m = 2 * g + j
                        st = (m == 0)
                        sp = (m == NP - 1)
                        for ti, dst in enumerate(q_ps + [k_ps, v_ps]):
                            # hi*hi for both d-tiles of the pair
                            nc.tensor.matmul(dst[:],
                                             w_sb[:, 4 * m:4 * m + 3:2, ti, :],
                                             x_r[:, j, 1:4:2, :],
                                             start=st, stop=False, perf_mode=DR)
                            # per d-tile: w_hi*x_lo + w_lo*x_hi
                            nc.tensor.matmul(dst[:],
                                             w_sb[:, 4 * m:4 * m + 2, ti, :],
                                             x_r[:, j, 0:2, :],
                                             start=False, stop=False,
                                             perf_mode=DR)
                            nc.tensor.matmul(dst[:],
                                             w_sb[:, 4 * m + 2:4 * m + 4, ti, :],
                                             x_r[:, j, 2:4, :],
                                             start=False, stop=sp, perf_mode=DR)
                    if sh == 1:
                        sh1_filler(g)

                # RoPE: dest = psum*[cos;cos] + swap(psum)*[-sin;sin].
                # fast=True splits the swap copies across ACT+DVE — used on
                # sh=1 where rope latency gates the phase-3 pipeline restart
                def rope(psum, dest, fast=False):
                    sw = rpool.tile([128, 512], f32, name="sw")
                    if fast:
                        nc.vector.tensor_copy(sw[0:64, :], psum[64:128, :])
                    else:
                        nc.scalar.copy(sw[0:64, :], psum[64:128, :])
                    nc.scalar.copy(sw[64:128, :], psum[0:64, :])
                    t1 = rpool.tile([128, 512], f32, name="t1")
                    nc.vector.tensor_tensor(t1[:], psum[:], cc_sb[:, s0:s0 + 512], op=mult)
                    t2 = rpool.tile([128, 512], f32, name="t2")
                    nc.gpsimd.tensor_tensor(t2[:], sw[:], ns_sb[:, s0:s0 + 512], op=mult)
                    nc.vector.tensor_tensor(dest, t1[:], t2[:], op=add)

                nc.vector.tensor_copy(v_et[:, s0:s0 + 512], v_ps[:])
                rope(q_ps[0], q_rot[0][:, s0:s0 + 512], fast=(sh == 1))
                rope(k_ps, k_rot[:, s0:s0 + 512], fast=(sh == 1))
                for h in range(1, NREP):
                    rope(q_ps[h], q_rot[h][:, s0:s0 + 512], fast=(sh == 1))

            # ---- phase 3: attention, flat (h, t, c) pipeline ----
            # wo for phase 4 loads during phase 3 (4MB, off the critical path)
            wo_sb = wts.tile([128, 8, D], f8, name="wo_sb", tag="wo", bufs=1)
            nc.sync.dma_start(wo_sb[:],
                              wo_d[:].rearrange("p (s f) -> p s f", f=D))

            # s-half-1 V transposes (feed o-matmuls for t>=4, reached later)
            for t in range(4, TT):
                transpose_v(t)

            PREFILL = 4
            emitted, consumed = 4, 0           # 4 seeds already emitted
            while consumed < len(jobs):
                if emitted < len(jobs) and emitted - consumed <= PREFILL:
                    emit_sc_exp(*jobs[emitted])
                    emitted += 1
                else:
                    consume_zo(*jobs[consumed])
                    consumed += 1

            if dbg:
                nc.sync.dma_start(dbg_d["qr0"][:], q_rot[0][:])
                nc.sync.dma_start(dbg_d["kr"][:], k_rot[:])
                nc.sync.dma_start(dbg_d["vte"][:], v_te[:])
                nc.sync.dma_start(
                    dbg_d["attc"][:],
                    att_c[:].rearrange("p a b -> p (a b)"))

            # ---- phase 4: output projection (fp8 DoubleRow over this core's
            # 512 att channels; psum is *S8*SW = 1024x, output copy scales back)
            for dg in range(DT // 2):
                last = (dg == DT // 2 - 1)
                out_sb = opool.tile([128, 2, S], bf16, name="out_sb")
                for dl in range(2):
                    do = 2 * dg + dl
                    dc = slice(128 * do, 128 * (do + 1))
                    op_ps = [ps.tile([128, 512], f32, tag="ps", name=f"op{c}")
                             for c in range(2)]
                    for c in range(2):
                        cs = slice(512 * c, 512 * (c + 1))
                        # per head: wo_hi*att_lo + wo_lo*att_hi
                        for hh in range(NREP):
                            nc.tensor.matmul(op_ps[c][:],
                                             wo_sb[:, 2 * hh:2 * hh + 2, dc],
                                             att_c[:, 2 * hh:2 * hh + 2, cs],
                                             start=(hh == 0), stop=False,
                                             perf_mode=DR)
                        # hi*hi for head pairs (0,1) and (2,3)
                        nc.tensor.matmul(op_ps[c][:], wo_sb[:, 0:3:2, dc],
                                         att_c[:, 1:4:2, cs],
                                         start=False, stop=False, perf_mode=DR)
                        nc.tensor.matmul(op_ps[c][:], wo_sb[:, 4:7:2, dc],
                                         att_c[:, 5:8:2, cs],
                                         start=False, stop=True, perf_mode=DR)
                        # copy each chunk as soon as its psum group closes
                        if c == 0:
                            nc.vector.tensor_scalar_mul(out_sb[:, dl, 0:512],
                                                        op_ps[0][:],
                                                        1.0 / (S8 * SW))
                        else:
                            nc.scalar.activation(out_sb[:, dl, 512:1024],
                                                 op_ps[1][:], Copy,
                                                 scale=1.0 / (S8 * SW))
                    if last:
                        # tail latency: ship each of the final tiles alone
                        nc.sync.dma_start(outT[dc, :], out_sb[:, dl, :])
                if not last:
                    nc.sync.dma_start(
                        outT[256 * dg:256 * (dg + 1), :].rearrange(
                            "(m p) s -> p m s", p=128),
                        out_sb[:])

        _body()

    nc.compile()
    return nc


def _to_f32r(x):
    """Host replica of the device fp32 -> fp32r conversion: round-to-nearest-
    even to an 11-bit mantissa (low 12 bits zeroed). Verified bit-exact against
    the DVE/DMA converters."""
    xi = np.ascontiguousarray(x, np.float32).view(np.uint32).astype(np.uint64)
    r = ((xi + 0x7FF + ((xi >> 12) & 1)) >> 12) << 12
    return (r & 0xFFFFFFFF).astype(np.uint32).view(np.float32)


def _split8(x):
    """fp8(e4m3) hi + lo residual split, shared scale. Returns (hi, lo)."""
    import ml_dtypes
    E4 = ml_dtypes.float8_e4m3
    hi = np.asarray(x, np.float32).astype(E4)
    lo = (np.asarray(x, np.float32) - hi.astype(np.float32)).astype(E4)
    return hi, lo


def ml_dtypes_bf16():
    import ml_dtypes
    return ml_dtypes.bfloat16


def kernel(**inputs):
    from concourse.bass_utils import run_bass_kernel_spmd

    x = np.asarray(inputs["x"], np.float32)                 # [1, S, D]
    cos = np.asarray(inputs["freqs_cos"], np.float32)       # [S, 64]
    sin = np.asarray(inputs["freqs_sin"], np.float32)       # [S, 64]
    wq = np.asarray(inputs["wq"], np.float32)               # [NH, HD, D]
    wk = np.asarray(inputs["wk"], np.float32)               # [NKV, HD, D]
    wv = np.asarray(inputs["wv"], np.float32)               # [NKV, HD, D]
    wo = np.asarray(inputs["wo"], np.float32)               # [D, D]
    input_pos = np.asarray(inputs["input_pos"]).astype(np.int64)  # [S]

    if "nc" not in _CACHE:
        _CACHE["nc"] = _build_nc()
    nc = _CACHE["nc"]

    perm = np.concatenate([np.arange(0, HD, 2), np.arange(1, HD, 2)])

    # x: [D, S] -> [p, sh, grp, m2, sub4, 512] with sub = (xl_a, xh_a, xl_b, xh_b)
    xT = x[0].T                                             # [D, S]
    xh, xl = _split8(xT)
    x4 = np.empty((DT, 2, 128, S), dtype=xh.dtype)          # [d-tile, lo/hi, p, s]
    x4[:, 1] = xh.reshape(DT, 128, S)
    x4[:, 0] = xl.reshape(DT, 128, S)
    # [d-tile, l/h, p, s] -> [m, m2, sub-lh, p, sh, 512] -> [p, sh, m, m2, sub, 512]
    xc = np.ascontiguousarray(
        x4.reshape(NP, 2, 2, 128, 2, 512)                   # [m, m2, lh, p, sh, 512]
        .transpose(3, 4, 0, 1, 2, 5)                        # [p, sh, m, m2, lh, 512]
        .reshape(128, 2 * NP * 2 * 2 * 512))
    # note: within a pair m the four subtiles must be (xl_a, xh_a, xl_b, xh_b)
    # -> order above is [m2(pair member), lh(lo,hi)] which flattens exactly so.

    def wsplit(wT):
        # [D, 128e] -> hi/lo interleaved [128p, 64 sub, 128e]
        h8, l8 = _split8(wT * SW)
        out = np.empty((128, 64, HD), dtype=h8.dtype)
        out[:, 0::2, :] = h8.reshape(DT, 128, HD).transpose(1, 0, 2)
        out[:, 1::2, :] = l8.reshape(DT, 128, HD).transpose(1, 0, 2)
        return out

    # cos/sin tables absorb the 1/SW unscale of q,k
    cc = np.ascontiguousarray(np.concatenate([cos.T, cos.T], 0)) / SW   # [128, S]
    ns = np.ascontiguousarray(np.concatenate([-sin.T, sin.T], 0)) / SW  # [128, S]
    # visibility adds +1 pre-exp where input_pos[t] <= input_pos[s]; for the
    # (spec-guaranteed) sorted arange fill only diagonal blocks are mixed.
    emaskd_t = np.empty((TT, 128, 128), np.float32)
    for t in range(TT):
        p = input_pos[128 * t:128 * (t + 1)]
        emaskd_t[t] = np.where(p[:, None] <= p[None, :], np.float32(np.e),
                               np.float32(1.0))
    emaskd = np.ascontiguousarray(
        emaskd_t.transpose(1, 0, 2).reshape(128, TT * 128))
    ident = np.eye(128, dtype=np.float32)
    cst = np.concatenate([cc, ns, ident], axis=1)
    cst16 = emaskd.astype(ml_dtypes_bf16())
    ones128 = np.ones((128, 128), np.float32)

    in_maps = []
    for g in range(NCORES):
        wq_g = wq[NREP * g:NREP * (g + 1)][:, perm, :]       # [4, 128, D]

        # wo rows for this core's 4 heads, hi/lo interleaved [128, 8 sub, D]
        wo_g = wo[:, NREP * HD * g:NREP * HD * (g + 1)].T    # [512, D]
        wh8, wl8 = _split8(wo_g * SW)
        wo_c = np.empty((128, 8, D), dtype=wh8.dtype)
        wo_c[:, 0::2, :] = wh8.reshape(NREP, 128, D).transpose(1, 0, 2)
        wo_c[:, 1::2, :] = wl8.reshape(NREP, 128, D).transpose(1, 0, 2)

        # [tensor(6), 128p, 64 sub, 128e] -> sub-major [128p, 64, 6, 128]
        wqkv = np.stack([wsplit(wq_g[j].T) for j in range(NREP)]
                        + [wsplit(wk[g][perm].T), wsplit(wv[g].T)])
        wqkv = np.ascontiguousarray(wqkv.transpose(1, 2, 0, 3))

        in_maps.append({
            "xc": xc,
            "wqkv_c": wqkv.reshape(128, 64 * 6 * HD),
            "wo_c": wo_c.reshape(128, 8 * D),
            "cst": cst,
            "cst16": cst16,
            "ones": _to_f32r(ones128),
        })

    res = run_bass_kernel_spmd(nc, in_maps, list(range(NCORES)))
    total = np.zeros((D, S), np.float32)
    for g in range(NCORES):
        total += res.results[g]["outT"].astype(np.float32)
    return np.ascontiguousarray(total.T[None])   # [1, S, D]


# revision 46
# speedup vs baseline: 1.1509x; 1.0304x over previous
"""Trainium2 Bass kernel for nn_AttentionSHA (dense transformer attention block).

Full inputs -> full output. Internally: tensor-parallel over heads across 8
NeuronCores (core g owns kv-head g and query heads 4g..4g+3; wo row-sharded),
host-side reduce of the 8 partial output projections (bf16 partials).

Performance: the QKV and WO projections run as fp8(e4m3) DoubleRow matmuls
(0.5 PE cycles per output row, 256-deep contraction per instruction) using a
hi+lo residual split of both operands for accuracy:
    a*b ~= a_hi*b_hi + (a_hi*b_lo + a_lo*b_hi)      [lo*lo dropped]
where v_hi = fp8(v), v_lo = fp8(v - v_hi) share one scale so all three
products accumulate in a single PSUM group: per pair of 128-deep contraction
tiles that is 3 DoubleRow instructions (vs 4 fp32r ones), i.e. 0.75x fp32r
cycles at ~10-bit effective mantissa (verified bit-exact vs ml_dtypes
emulation on device). The attention core (scores / exp / z / PV) stays fp32r.

DMA discipline: every DMACopy costs ~625ns on the shared HWDGE descriptor
generator in issue order, so transfers are batched into few chunk-contiguous
DMAs (x is laid out on host so each (s-half, m-group) chunk is one
contiguous 4KB/partition run).

Math notes (validated against the reference in fp64/fp32 numpy):
  - The reference adds a 0/1 causal mask *before* softmax (no -inf masking) and
    runs softmax over the full MAXSEQ=2048 cache axis where positions >= S hold
    zero k/v. Softmax without max-subtraction is exact here (scores are in
    [-17, 18]), so:  out = sum_t exp(sc_t)*m_t*v_t / (sum_t exp(sc_t)*m_t + 1024)
    with m_t = e if visible else 1, and +1024 = (MAXSEQ - S) zero-score tail.
    The e-factor for fully-visible regions folds into the Exp bias
    (exp(x + 1) = e*exp(x)); only the 128x128 diagonal blocks need a mask mult.
  - RoPE is applied via host-permuted weight rows (even channels then odd), a
    partition-half swap, and two multiply-adds against [cos;cos] / [-sin;sin].
  - Scales: wq/wk/wv stored *64 (cos/sin tables pre-divided by 64 unscale q,k
    in the rope); v flows *64 into PV, fixed by z' = (z+1024)*4 so the
    normalized att comes out *16 = the fp8 storage scale for the WO moving
    operand; wo stored *64; final psum *1024 -> output copy scales by 2^-10.
"""
import numpy as np
from contextlib import ExitStack

S = 1024
D = 4096
NH = 32
NKV = 8
HD = 128
NREP = NH // NKV          # 4
MAXSEQ = 2048
NCORES = 8
DT = D // 128             # 32 d-tiles
TT = S // 128             # 8 t-tiles
NP = DT // 2              # 16 d-tile pairs (DoubleRow steps)
NG = NP // 2              # 8 x-chunk groups per s-half (2 pairs each)

SW = 64.0                 # weight fp8 scale (wq/wk/wv/wo)
S8 = 16.0                 # att fp8 scale

_CACHE = {}


def _build_nc(dbg=False):
    import concourse.bacc as bacc
    import concourse.mybir as mybir
    import concourse.tile as tile

    f32 = mybir.dt.float32
    f32r = mybir.dt.float32r
    bf16 = mybir.dt.bfloat16
    f8 = mybir.dt.float8e4
    Exp = mybir.ActivationFunctionType.Exp
    Copy = mybir.ActivationFunctionType.Copy
    mult = mybir.AluOpType.mult
    add = mybir.AluOpType.add
    sub = mybir.AluOpType.subtract
    DR = mybir.MatmulPerfMode.DoubleRow

    nc = bacc.Bacc("TRN2", target_bir_lowering=False, debug=False,
                   num_devices=NCORES)

    # x hi/lo interleaved, chunk-contiguous:
    # [128, sh(2), grp(8), m2(2), sub(4), 512]; sub = (xl_a, xh_a, xl_b, xh_b)
    xc_d = nc.dram_tensor("xc", [128, 2 * NG * 2 * 4 * 512], f8,
                          kind="ExternalInput")
    # qkv weights hi/lo interleaved, sub-major so any subtile span is one
    # contiguous DMA: [128, 64 sub, 6 tensors (q0..q3,k,v), 128]
    wqkv_d = nc.dram_tensor("wqkv_c", [128, 64 * 6 * HD], f8,
                            kind="ExternalInput")
    # wo hi/lo interleaved over the head axis: [128, 8 sub, D]
    wo_d = nc.dram_tensor("wo_c", [128, 8 * D], f8, kind="ExternalInput")
    # merged constants: cc | ns | ident (f32); emaskd (bf16)
    cst_d = nc.dram_tensor("cst", [128, 2 * S + 128], f32, kind="ExternalInput")
    cst16_d = nc.dram_tensor("cst16", [128, TT * 128], bf16,
                             kind="ExternalInput")
    ones_d = nc.dram_tensor("ones", [128, 128], f32r, kind="ExternalInput")
    outT = nc.dram_tensor("outT", [D, S], bf16, kind="ExternalOutput")
    if dbg:
        dbg_d = {k: nc.dram_tensor(k, shp, dt, kind="ExternalOutput")
                 for k, shp, dt in [
                     ("qr0", [128, S], f32r), ("kr", [128, S], f32r),
                     ("vte", [128, S], f32r), ("ex00", [128, S], f32r),
                     ("ex05", [128, S], f32r), ("attc", [128, 8 * S], f8),
                     ("zz", [128, S], f32), ("rzz", [128, S], f32)]}

    with tile.TileContext(nc) as tc, ExitStack() as ctx:
        const = ctx.enter_context(tc.tile_pool(name="const", bufs=1))
        wts = ctx.enter_context(tc.tile_pool(name="wts", bufs=6))
        xpool = ctx.enter_context(tc.tile_pool(name="xpool", bufs=4))
        rpool = ctx.enter_context(tc.tile_pool(name="rpool", bufs=3))
        qkv = ctx.enter_context(tc.tile_pool(name="qkv", bufs=1))
        hs = ctx.enter_context(tc.tile_pool(name="hs", bufs=4))
        epool = ctx.enter_context(tc.tile_pool(name="epool", bufs=5))
        zpool = ctx.enter_context(tc.tile_pool(name="zpool", bufs=1))
        apool = ctx.enter_context(tc.tile_pool(name="apool", bufs=3))
        opool = ctx.enter_context(tc.tile_pool(name="opool", bufs=3))
        ps = ctx.enter_context(tc.tile_pool(name="ps", bufs=8, space="PSUM"))

        def _body():
            cst_sb = const.tile([128, 2 * S + 128], f32)
            cc_sb = cst_sb[:, 0:S]
            ns_sb = cst_sb[:, S:2 * S]
            ident_sb = cst_sb[:, 2 * S:]
            cst16_sb = const.tile([128, TT * 128], bf16)
            emaskd_sb = cst16_sb[:, 0:TT * 128]
            ones_sb = const.tile([128, 128], f32r)

            w_sb = wts.tile([128, 64, 6, HD], f8, name="w_sb", tag="w16",
                            bufs=1)

            def load_w_span(m0, mn):
                s0, s1 = 4 * m0, 4 * (m0 + mn)
                c0, c1 = s0 * 6 * HD, s1 * 6 * HD
                nc.sync.dma_start(
                    w_sb[:, s0:s1, :, :],
                    wqkv_d[:, c0:c1].rearrange("p (s t f) -> p s t f",
                                               t=6, f=HD))

            # just-in-time pair spans, one x-chunk group of lookahead; the
            # first span (pair 0) is hoisted before the loop for fast start
            _wb = {0: [(1, 1), (2, 2)]}
            _wb.update({g: [(2 * g + 2, 2)] for g in range(1, NG - 1)})

            # ---- phase 1: QKV projections (fp8 DoubleRow) + RoPE ----
            q_rot = [hs.tile([128, S], f32r, name=f"q_rot{h}", tag="hs")
                     for h in range(NREP)]                      # per head [e, s]
            k_rot = qkv.tile([128, S], f32r)                    # [e, t]
            v_et = qkv.tile([128, S], f32)                      # [e, t] pre-transpose
            v_te = qkv.tile([128, TT * 128], f32r)              # tile t: [t-part, e]
            inv_sqrt_hd = float(1.0 / np.sqrt(HD))

            # ---------- phase 3 pipeline helpers (used from sh=1 too) ----------
            expm_tiles = {}
            zo_ps = {}

            def emit_sc_exp(h, t, c):
                dlo, dhi = 128 * t, 128 * (t + 1)
                lo, hi = 512 * c, 512 * (c + 1)
                if (h, t) not in expm_tiles:
                    expm_tiles[(h, t)] = epool.tile([128, S], f32r, name="expm")
                expm = expm_tiles[(h, t)]
                sc = ps.tile([128, 512], f32, tag="ps", name="sc")
                nc.tensor.matmul(sc[:], k_rot[:, dlo:dhi],
                                 q_rot[h][:, lo:hi], start=True, stop=True)
                if dlo >= hi:
                    # fully invisible: plain exp
                    nc.scalar.activation(expm[:, lo:hi], sc[:], Exp,
                                         scale=inv_sqrt_hd)
                elif dhi <= lo:
                    # fully visible: exp(x + 1) = e * exp(x)
                    nc.scalar.activation(expm[:, lo:hi], sc[:], Exp,
                                         scale=inv_sqrt_hd, bias=1.0)
                else:
                    # diagonal block inside this chunk: one exp call, then the
                    # mask factors applied in-place (diag x emaskd on GpSimd;
                    # visible remainder x e on GpSimd)
                    nc.scalar.activation(expm[:, lo:hi], sc[:], Exp,
                                         scale=inv_sqrt_hd)
                    nc.gpsimd.tensor_tensor(
                        expm[:, dlo:dhi], expm[:, dlo:dhi],
                        emaskd_sb[:, 128 * t:128 * (t + 1)], op=mult)
                    if dhi < hi:
                        nc.gpsimd.tensor_scalar_mul(
                            expm[:, dhi:hi], expm[:, dhi:hi], float(np.e))

            att_c = apool.tile([128, 8, S], f8, name="att_c", tag="attc", bufs=1)
            z_sb = zpool.tile([128, S], f32, name="z_sb")
            rz = zpool.tile([128, S], f32, name="rz")

            def consume_zo(h, t, c):
                if (h, c) not in zo_ps:
                    zo_ps[(h, c)] = [ps.tile([128, 512], f32, tag="ps",
                                             name=f"zo{h}_{c}{i}")
                                     for i in range(2)]
                zp, op = zo_ps[(h, c)]
                cs = slice(512 * c, 512 * (c + 1))
                expm = expm_tiles[(h, t)]
                nc.tensor.matmul(zp[:], ones_sb[:], expm[:, cs],
                                 start=(t == 0), stop=(t == TT - 1))
                nc.tensor.matmul(op[:], v_te[:, 128 * t:128 * (t + 1)],
                                 expm[:, cs], start=(t == 0), stop=(t == TT - 1))
                if t == TT - 1:
                    # z' = (z + tail) * (SW/S8); att = o * (1/z') comes out *S8
                    nc.vector.tensor_scalar(z_sb[:, cs], zp[:],
                                            float(MAXSEQ - S), SW / S8,
                                            op0=add, op1=mult)
                    nc.vector.reciprocal(rz[:, cs], z_sb[:, cs])
                    at = apool.tile([128, 512], f32, name="at", tag="at")
                    nc.vector.tensor_tensor(at[:], op[:], rz[:, cs], op=mult)
                    # hi = fp8(at) on DVE; lo = at - hi on GpSimd
                    nc.vector.tensor_copy(att_c[:, 2 * h + 1, cs], at[:])
                    nc.gpsimd.tensor_tensor(att_c[:, 2 * h, cs], at[:],
                                            att_c[:, 2 * h + 1, cs], op=sub)
                    if dbg and h == 0 and c == 1:
                        nc.sync.dma_start(dbg_d["zz"][:], z_sb[:])
                        nc.sync.dma_start(dbg_d["rzz"][:], rz[:])
                if dbg and (h, t) == (0, 0) and c == 1:
                    nc.sync.dma_start(dbg_d["ex00"][:], expm[:])
                if dbg and (h, t) == (0, 5) and c == 1:
                    nc.sync.dma_start(dbg_d["ex05"][:], expm[:])

            def transpose_v(t):
                tr = ps.tile([128, 128], f32, tag="ps", name="tr")
                nc.tensor.transpose(tr[:], v_et[:, 128 * t:128 * (t + 1)],
                                    ident_sb[:])
                nc.vector.tensor_copy(v_te[:, 128 * t:128 * (t + 1)], tr[:])

            # jobs: head 0 c=0 for t<4 is feasible right after sh=0 rope --
            # those four (the seeds) are emitted inside the sh=1 DR stream.
            jobs = [(0, t, 0) for t in range(4)]
            for h in range(NREP):
                for t in range(TT):
                    for c in range(2):
                        if (h, t, c) not in jobs:
                            jobs.append((h, t, c))

            # interleaved into the sh=1 stream at m-group boundaries:
            # [transpose_v(0..3) at g=0,1 | seeds at g=2..5]
            def sh1_filler(g):
                if g < 2:
                    transpose_v(2 * g)
                    transpose_v(2 * g + 1)
                elif g < 6:
                    emit_sc_exp(*jobs[g - 2])

            # x chunks are DMA'd two groups ahead of consumption
            x_tiles = {}

            def prime_x(idx, split=False):
                if idx >= 2 * NG:
                    return
                x_r = x_tiles[idx] = xpool.tile([128, 2, 4, 512], f8,
                                                name="x_r")
                if split:
                    # two half-chunk DMAs: the first pair's matmuls gate on
                    # a 0.7us transfer instead of 1.5us
                    for j in range(2):
                        nc.sync.dma_start(
                            x_r[:, j, :, :],
                            xc_d[:, idx * 4096 + j * 2048:
                                 idx * 4096 + (j + 1) * 2048].rearrange(
                                "p (s f) -> p s f", f=512))
                        if j == 0:
                            load_w_span(0, 1)
                else:
                    nc.sync.dma_start(
                        x_r[:],
                        xc_d[:, idx * 4096:(idx + 1) * 4096].rearrange(
                            "p (m s f) -> p m s f", m=2, f=512))

            prime_x(0, split=True)
            prime_x(1)
            for sh in range(2):
                s0 = 512 * sh
                q_ps = [ps.tile([128, 512], f32, tag="ps", name=f"q_ps{sh}_{h}")
                        for h in range(NREP)]
                k_ps = ps.tile([128, 512], f32, tag="ps", name=f"k_ps{sh}")
                v_ps = ps.tile([128, 512], f32, tag="ps", name=f"v_ps{sh}")
                for g in range(NG):
                    x_r = x_tiles.pop(sh * NG + g)
                    prime_x(sh * NG + g + 2)
                    if sh == 0 and g in _wb:
                        for span in _wb[g]:
                            load_w_span(*span)
                    if sh == 0 and g == 6:
                        nc.sync.dma_start(cst_sb[:], cst_d[:])
                    if sh == 1 and g == 0:
                        nc.sync.dma_start(cst16_sb[:], cst16_d[:])
                        nc.sync.dma_start(ones_sb[:], ones_d[:])
                    for j in range(2):
                        m = 2 * g + j
                        st = (m == 0)
                        sp = (m == NP - 1)
                        for ti, dst in enumerate(q_ps + [k_ps, v_ps]):
                            # hi*hi for both d-tiles of the pair
                            nc.tensor.matmul(dst[:],
                                             w_sb[:, 4 * m:4 * m + 3:2, ti, :],
                                             x_r[:, j, 1:4:2, :],
                                             start=st, stop=False, perf_mode=DR)
                            # per d-tile: w_hi*x_lo + w_lo*x_hi
                            nc.tensor.matmul(dst[:],
                                             w_sb[:, 4 * m:4 * m + 2, ti, :],
                                             x_r[:, j, 0:2, :],
                                             start=False, stop=False,
                                             perf_mode=DR)
                            nc.tensor.matmul(dst[:],
                                             w_sb[:, 4 * m + 2:4 * m + 4, ti, :],
                                             x_r[:, j, 2:4, :],
                                             start=False, stop=sp, perf_mode=DR)
                    if sh == 1:
                        sh1_filler(g)

                # RoPE: dest = psum*[cos;cos] + swap(psum)*[-sin;sin].
                # fast=True splits the swap copies across ACT+DVE — used on
                # sh=1 where rope latency gates the phase-3 pipeline restart
                def rope(psum, dest, fast=False):
                    sw = rpool.tile([128, 512], f32, name="sw")
                    if fast:
                        nc.vector.tensor_copy(sw[0:64, :], psum[64:128, :])
                    else:
                        nc.scalar.copy(sw[0:64, :], psum[64:128, :])
                    nc.scalar.copy(sw[64:128, :], psum[0:64, :])
                    t1 = rpool.tile([128, 512], f32, name="t1")
                    nc.vector.tensor_tensor(t1[:], psum[:], cc_sb[:, s0:s0 + 512], op=mult)
                    t2 = rpool.tile([128, 512], f32, name="t2")
                    nc.gpsimd.tensor_tensor(t2[:], sw[:], ns_sb[:, s0:s0 + 512], op=mult)
                    nc.vector.tensor_tensor(dest, t1[:], t2[:], op=add)

                nc.vector.tensor_copy(v_et[:, s0:s0 + 512], v_ps[:])
                rope(q_ps[0], q_rot[0][:, s0:s0 + 512], fast=(sh == 1))
                rope(k_ps, k_rot[:, s0:s0 + 512], fast=(sh == 1))
                for h in range(1, NREP):
                    rope(q_ps[h], q_rot[h][:, s0:s0 + 512], fast=(sh == 1))

            # ---- phase 3: attention, flat (h, t, c) pipeline ----
            # wo for phase 4 loads during phase 3 (4MB, off the critical path)
            wo_sb = wts.tile([128, 8, D], f8, name="wo_sb", tag="wo", bufs=1)
            nc.sync.dma_start(wo_sb[:],
                              wo_d[:].rearrange("p (s f) -> p s f", f=D))

            # s-half-1 V transposes (feed o-matmuls for t>=4, reached later)
            for t in range(4, TT):
                transpose_v(t)

            PREFILL = 4
            emitted, consumed = 4, 0           # 4 seeds already emitted
            while consumed < len(jobs):
                if emitted < len(jobs) and emitted - consumed <= PREFILL:
                    emit_sc_exp(*jobs[emitted])
                    emitted += 1
                else:
                    consume_zo(*jobs[consumed])
                    consumed += 1

            if dbg:
                nc.sync.dma_start(dbg_d["qr0"][:], q_rot[0][:])
                nc.sync.dma_start(dbg_d["kr"][:], k_rot[:])
                nc.sync.dma_start(dbg_d["vte"][:], v_te[:])
                nc.sync.dma_start(
                    dbg_d["attc"][:],
                    att_c[:].rearrange("p a b -> p (a b)"))

            # ---- phase 4: output projection (fp8 DoubleRow over this core's
            # 512 att channels; psum is *S8*SW = 1024x, output copy scales back)
            for dg in range(DT // 2):
                last = (dg == DT // 2 - 1)
                out_sb = opool.tile([128, 2, S], bf16, name="out_sb")
                for dl in range(2):
                    do = 2 * dg + dl
                    dc = slice(128 * do, 128 * (do + 1))
                    op_ps = [ps.tile([128, 512], f32, tag="ps", name=f"op{c}")
                             for c in range(2)]
                    for c in range(2):
                        cs = slice(512 * c, 512 * (c + 1))
                        # head order (corr_0, corr_1, hi01, corr_2, corr_3,
                        # hi23) so early instructions need only early heads
                        for hh in range(2):
                            nc.tensor.matmul(op_ps[c][:],
                                             wo_sb[:, 2 * hh:2 * hh + 2, dc],
                                             att_c[:, 2 * hh:2 * hh + 2, cs],
                                             start=(hh == 0), stop=False,
                                             perf_mode=DR)
                        nc.tensor.matmul(op_ps[c][:], wo_sb[:, 0:3:2, dc],
                                         att_c[:, 1:4:2, cs],
                                         start=False, stop=False, perf_mode=DR)
                        for hh in range(2, NREP):
                            nc.tensor.matmul(op_ps[c][:],
                                             wo_sb[:, 2 * hh:2 * hh + 2, dc],
                                             att_c[:, 2 * hh:2 * hh + 2, cs],
                                             start=False, stop=False,
                                             perf_mode=DR)
                        nc.tensor.matmul(op_ps[c][:], wo_sb[:, 4:7:2, dc],
                                         att_c[:, 5:8:2, cs],
                                         start=False, stop=True, perf_mode=DR)
                        # copy each chunk as soon as its psum group closes
                        if c == 0:
                            nc.vector.tensor_scalar_mul(out_sb[:, dl, 0:512],
                                                        op_ps[0][:],
                                                        1.0 / (S8 * SW))
                        else:
                            nc.scalar.activation(out_sb[:, dl, 512:1024],
                                                 op_ps[1][:], Copy,
                                                 scale=1.0 / (S8 * SW))
                    if last:
                        # tail latency: ship each of the final tiles alone
                        nc.sync.dma_start(outT[dc, :], out_sb[:, dl, :])
                if not last:
                    nc.sync.dma_start(
                        outT[256 * dg:256 * (dg + 1), :].rearrange(
                            "(m p) s -> p m s", p=128),
                        out_sb[:])

        _body()

    nc.compile()
    return nc


def _to_f32r(x):
    """Host replica of the device fp32 -> fp32r conversion: round-to-nearest-
    even to an 11-bit mantissa (low 12 bits zeroed). Verified bit-exact against
    the DVE/DMA converters."""
    xi = np.ascontiguousarray(x, np.float32).view(np.uint32).astype(np.uint64)
    r = ((xi + 0x7FF + ((xi >> 12) & 1)) >> 12) << 12
    return (r & 0xFFFFFFFF).astype(np.uint32).view(np.float32)


def _split8(x):
    """fp8(e4m3) hi + lo residual split, shared scale. Returns (hi, lo)."""
    import ml_dtypes
    E4 = ml_dtypes.float8_e4m3
    hi = np.asarray(x, np.float32).astype(E4)
    lo = (np.asarray(x, np.float32) - hi.astype(np.float32)).astype(E4)
    return hi, lo


def ml_dtypes_bf16():
    import ml_dtypes
    return ml_dtypes.bfloat16


def kernel(**inputs):
    from concourse.bass_utils import run_bass_kernel_spmd

    x = np.asarray(inputs["x"], np.float32)                 # [1, S, D]
    cos = np.asarray(inputs["freqs_cos"], np.float32)       # [S, 64]
    sin = np.asarray(inputs["freqs_sin"], np.float32)       # [S, 64]
    wq = np.asarray(inputs["wq"], np.float32)               # [NH, HD, D]
    wk = np.asarray(inputs["wk"], np.float32)               # [NKV, HD, D]
    wv = np.asarray(inputs["wv"], np.float32)               # [NKV, HD, D]
    wo = np.asarray(inputs["wo"], np.float32)               # [D, D]
    input_pos = np.asarray(inputs["input_pos"]).astype(np.int64)  # [S]

    if "nc" not in _CACHE:
        _CACHE["nc"] = _build_nc()
    nc = _CACHE["nc"]

    perm = np.concatenate([np.arange(0, HD, 2), np.arange(1, HD, 2)])

    # x: [D, S] -> [p, sh, grp, m2, sub4, 512] with sub = (xl_a, xh_a, xl_b, xh_b)
    xT = x[0].T                                             # [D, S]
    xh, xl = _split8(xT)
    x4 = np.empty((DT, 2, 128, S), dtype=xh.dtype)          # [d-tile, lo/hi, p, s]
    x4[:, 1] = xh.reshape(DT, 128, S)
    x4[:, 0] = xl.reshape(DT, 128, S)
    # [d-tile, l/h, p, s] -> [m, m2, sub-lh, p, sh, 512] -> [p, sh, m, m2, sub, 512]
    xc = np.ascontiguousarray(
        x4.reshape(NP, 2, 2, 128, 2, 512)                   # [m, m2, lh, p, sh, 512]
        .transpose(3, 4, 0, 1, 2, 5)                        # [p, sh, m, m2, lh, 512]
        .reshape(128, 2 * NP * 2 * 2 * 512))
    # note: within a pair m the four subtiles must be (xl_a, xh_a, xl_b, xh_b)
    # -> order above is [m2(pair member), lh(lo,hi)] which flattens exactly so.

    def wsplit(wT):
        # [D, 128e] -> hi/lo interleaved [128p, 64 sub, 128e]
        h8, l8 = _split8(wT * SW)
        out = np.empty((128, 64, HD), dtype=h8.dtype)
        out[:, 0::2, :] = h8.reshape(DT, 128, HD).transpose(1, 0, 2)
        out[:, 1::2, :] = l8.reshape(DT, 128, HD).transpose(1, 0, 2)
        return out

    # cos/sin tables absorb the 1/SW unscale of q,k
    cc = np.ascontiguousarray(np.concatenate([cos.T, cos.T], 0)) / SW   # [128, S]
    ns = np.ascontiguousarray(np.concatenate([-sin.T, sin.T], 0)) / SW  # [128, S]
    # visibility adds +1 pre-exp where input_pos[t] <= input_pos[s]; for the
    # (spec-guaranteed) sorted arange fill only diagonal blocks are mixed.
    emaskd_t = np.empty((TT, 128, 128), np.float32)
    for t in range(TT):
        p = input_pos[128 * t:128 * (t + 1)]
        emaskd_t[t] = np.where(p[:, None] <= p[None, :], np.float32(np.e),
                               np.float32(1.0))
    emaskd = np.ascontiguousarray(
        emaskd_t.transpose(1, 0, 2).reshape(128, TT * 128))
    ident = np.eye(128, dtype=np.float32)
    cst = np.concatenate([cc, ns, ident], axis=1)
    cst16 = emaskd.astype(ml_dtypes_bf16())
    ones128 = np.ones((128, 128), np.float32)

    in_maps = []
    for g in range(NCORES):
        wq_g = wq[NREP * g:NREP * (g + 1)][:, perm, :]       # [4, 128, D]

        # wo rows for this core's 4 heads, hi/lo interleaved [128, 8 sub, D]
        wo_g = wo[:, NREP * HD * g:NREP * HD * (g + 1)].T    # [512, D]
        wh8, wl8 = _split8(wo_g * SW)
        wo_c = np.empty((128, 8, D), dtype=wh8.dtype)
        wo_c[:, 0::2, :] = wh8.reshape(NREP, 128, D).transpose(1, 0, 2)
        wo_c[:, 1::2, :] = wl8.reshape(NREP, 128, D).transpose(1, 0, 2)

        # [tensor(6), 128p, 64 sub, 128e] -> sub-major [128p, 64, 6, 128]
        wqkv = np.stack([wsplit(wq_g[j].T) for j in range(NREP)]
                        + [wsplit(wk[g][perm].T), wsplit(wv[g].T)])
        wqkv = np.ascontiguousarray(wqkv.transpose(1, 2, 0, 3))

        in_maps.append({
            "xc": xc,
            "wqkv_c": wqkv.reshape(128, 64 * 6 * HD),
            "wo_c": wo_c.reshape(128, 8 * D),
            "cst": cst,
            "cst16": cst16,
            "ones": _to_f32r(ones128),
        })

    res = run_bass_kernel_spmd(nc, in_maps, list(range(NCORES)))
    total = np.zeros((D, S), np.float32)
    for g in range(NCORES):
        total += res.results[g]["outT"].astype(np.float32)
    return np.ascontiguousarray(total.T[None])   # [1, S, D]
